# revision 30
# baseline (speedup 1.0000x reference)
"""FISTA sparse-coding encoder kernel for Trainium2 (8 NeuronCores).

Problem: x [2,10,20480] f32, Drr/Dtheta [40] f32.
  D = normalized dictionary [10, 161]
  A = I - D^T D / L,  DtY = D^T Y / L,  lam = gamma / L
  40 FISTA iterations: xn = softshrink(A @ y + DtY); y = xn + m (xn - x_old)
  output sparsecode [2, 161, 20480].

Design ("fold-8"):
  - Data-parallel over columns: Y reshaped to [10, 40960]; 5120 columns/core.
  - u-form + momentum identity: u_i = A x_i + DtY;
    A y_i + DtY = (1+m) u_i - m u_{i-1}.
  - Output-row split [118 | 43]; Y is glued into the HEAD contraction k-tile
    (state rows 118:128), so the head DVE write (rows 0:118) never touches it.
  - Tail folding: group pairs (2p, 2p+1) share one elementwise pack. Matmul
    PSUM stays at partition base 0 (column-tiled PE writes are rejected by
    walrus); the fold happens via DMA: raw tail u is DMA-evacuated into a
    ping-pong SBUF pair tile at partition offsets {0, 64}. One 7-stage DVE
    op (w = C0*u + C1*u_prev, then clamp-shrink) covers both groups.
    => 8 elementwise packs/iter instead of 10, and the pair packs carry
    their own momentum history (no ACT work for them).
  - Heads + last odd group: classic path (PSUM in0 + ACT-evacuated -m*u).
  - Tail state is column-folded [107, 3072]; odd groups' matmul operands
    (weights dup + state) live at partition base 64 (row-tiled PE reads).
  - State zero-initialised => iteration 0 is a regular iteration
    (u_0 = A*0 + DtY) with a Src1-less fused op; no special weights.
  - All matmul operands float32r (full-rate fp32 PE path).
"""

import numpy as np

# ---------------------------------------------------------------- constants
B, T, N_POLES, P = 2, 10, 40, 20480
MAX_ITER = 40
GAMMA = 0.01
K = 4 * N_POLES + 1          # 161
NCORES = 8
NCOLS = B * P // NCORES      # 5120 columns per core
BLK = 512                    # matmul free dim (one PSUM bank)
GRP = 1024                   # elementwise pack width (2 banks)
NGRP = NCOLS // GRP          # 5
KH = 118                     # head output rows
KT = K - KH                  # 43 tail output rows
CON1 = KH + T                # 128 = head k-tile: x_head(118) + Y(10)
FOLD = 64                    # partition offset of odd-group tails
TROWS = FOLD + KT            # 107 rows in folded tail tiles
NPAIR = NGRP // 2            # 2 full pairs; group 4 is dual-x
TCOLS = NPAIR * GRP          # 2048 folded tail columns (pairs only)
G4 = NGRP - 1                # the dual-x group
W4C = 3 * K                  # 483 = per-iteration dual-x weight columns

_cache = {}


# ------------------------------------------------------------ custom DVE ops
def _register_shrink_ops():
    """shrink(w) = w - clamp(w, -C2, C2).

    op_mom   : w = in0*s0 + in1          (in1 pre-scaled by ACT: -m*u_prev)
    op_mom2  : w = in0*s0 + in1*s1       (both raw u copies in SBUF)
    op_first : w = in0*s0                (iteration 0)
    """
    import concourse.dve_ops as dve_ops
    from concourse.dve_spec import Spec, Src0, Src1, C0, C1, C2, Zero, minn, maxx, lower
    from concourse.dve_spec import _has_src1 as has_src1
    from concourse.dve_uop import DveOpSpec

    def reg(name, body, ref):
        if any(op.name == name for op in dve_ops.OPS):
            return next(op for op in dve_ops.OPS if op.name == name)
        spec = Spec(body=body, reference=ref)
        op = dve_ops.DveOp(name, spec, subdim=False, uops_sha={})
        dve_ops.OPS.append(op)
        dve_ops.CUSTOM_DVE_SPECS[name] = spec
        dve_ops._SUB_OPCODE_FOR_NAME[name] = (
            dve_ops._CUSTOM_DVE_ROW_BASE + len(dve_ops.OPS) - 1
        )
        for ver in ("v3", "v4"):
            compiled = DveOpSpec(
                name=name,
                opcode=dve_ops.get_dve_sub_opcode(name),
                uops=lower(spec, ver=ver),
                rd1_en=has_src1(spec),
            )
            op.uops_sha[ver] = compiled.sha(ver)
        return op

    def shrink_ref(ww, imm2):
        return (ww - np.maximum(np.minimum(ww, imm2), -imm2)).astype(np.float32)

    w1 = Src0 * C0 + Src1
    op_mom = reg(
        "ANT_SHRINKCLAMP_MOM",
        w1 - maxx(minn(w1, C2), Zero - C2),
        lambda in0, in1, s0=1.0, s1=0.0, imm2=0.0: shrink_ref(in0 * s0 + in1, imm2),
    )
    w2 = Src0 * C0 + Src1 * C1
    op_mom2 = reg(
        "ANT_SHRINKCLAMP_MOM2",
        w2 - maxx(minn(w2, C2), Zero - C2),
        lambda in0, in1, s0=1.0, s1=0.0, imm2=0.0: shrink_ref(
            in0 * s0 + in1 * s1, imm2),
    )
    w0 = Src0 * C0
    op_first = reg(
        "ANT_SHRINKCLAMP_FIRST",
        w0 - maxx(minn(w0, C2), Zero - C2),
        lambda in0, s0=1.0, s1=0.0, imm2=0.0: shrink_ref(in0 * s0, imm2),
    )
    return op_mom, op_mom2, op_first


# ------------------------------------------------------------ host constants
def _host_constants(Drr, Dtheta):
    r = Drr.astype(np.float64)
    th = Dtheta.astype(np.float64)
    i = np.arange(T, dtype=np.float64)[:, None]
    pr = r[None, :] ** i
    sgn = np.where(np.arange(T)[:, None] % 2 == 0, 1.0, -1.0)
    c = np.cos(i * th[None, :])
    s = np.sin(i * th[None, :])
    ones = np.ones((T, 1))
    dic = np.concatenate([ones, pr * c, sgn * pr * c, pr * s, sgn * pr * s], axis=1)
    G = np.linalg.norm(dic, axis=0)
    G = np.where(G == 0, np.sqrt(float(T)), G)
    D = dic / G                                 # [T, K] float64

    DtD = D.T @ D
    L = float(np.linalg.norm(DtD))              # Frobenius
    A = np.eye(K) - DtD / L                     # [K, K]
    lam = float(GAMMA / L)

    Aaug = np.concatenate([A.T, D / L], axis=0).astype(np.float32)  # [171, K]

    ms = []
    t = 1.0
    for _ in range(MAX_ITER):
        t_new = (1.0 + np.sqrt(1.0 + 4.0 * t * t)) / 2.0
        ms.append((t - 1.0) / t_new)
        t = t_new
    return Aaug, lam, ms


# ------------------------------------------------------------- bass program
def _build_program():
    import concourse.mybir as mybir
    import concourse.tile as tile
    from concourse import bacc

    op_mom, op_mom2, op_first = _register_shrink_ops()

    f32 = mybir.dt.float32
    f32r = mybir.dt.float32r

    nc = bacc.Bacc("TRN2", target_bir_lowering=False, debug=False,
                   num_devices=NCORES)

    ycols = nc.dram_tensor("ycols", [T, NCOLS], f32, kind="ExternalInput")
    d_w1h = nc.dram_tensor("w1h", [CON1, KH], f32, kind="ExternalInput")
    d_w1t = nc.dram_tensor("w1t", [CON1, KT], f32, kind="ExternalInput")
    d_w2h = nc.dram_tensor("w2h", [TROWS, KH], f32, kind="ExternalInput")
    d_w2t = nc.dram_tensor("w2t", [TROWS, KT], f32, kind="ExternalInput")
    d_w4 = nc.dram_tensor("w4", [CON1, W4C * MAX_ITER], f32,
                          kind="ExternalInput")
    out = nc.dram_tensor("out", [K, NCOLS], f32, kind="ExternalOutput")

    lam, ms = _cache["consts_meta"]

    with tile.TileContext(nc) as tc:
        with (
            tc.tile_pool(name="state", bufs=1) as st,
            tc.tile_pool(name="wts", bufs=1) as wts,
            tc.tile_pool(name="psH", bufs=3, space="PSUM") as psH,
            tc.tile_pool(name="psT", bufs=1, space="PSUM") as psT,
        ):
            # ---- persistent state -------------------------------------
            # xH rows 0:118 = head state, rows 118:128 = Y glue (static)
            xH = [st.tile([CON1, NCOLS], f32r, tag=f"xH{b}", name=f"xH{b}")
                  for b in range(2)]
            # folded tail state: rows 0:43 even groups, 64:107 odd groups
            xT = [st.tile([TROWS, TCOLS], f32r, tag=f"xT{b}", name=f"xT{b}")
                  for b in range(2)]
            # double-buffered so the evacuation never waits on the fused op
            uoH = [st.tile([KH, NCOLS], f32, tag=f"uoH{b}", name=f"uoH{b}")
                   for b in range(2)]
            # group-4 dual-x tail state: x_i at rows 0:43 (even i) or
            # 64:107 (odd i); the other slot holds x_{i-1}
            tg4 = st.tile([TROWS, GRP], f32r, tag="tg4", name="tg4")
            # raw tail-u pair copies (ping-pong momentum history)
            upT = [st.tile([TROWS, NPAIR * GRP], f32, tag=f"upT{b}",
                           name=f"upT{b}") for b in range(2)]
            # staging for the odd-group partition shift (engine writes are
            # lane-locked; only DMA can move partitions 0:43 -> 64:107)
            ush = [st.tile([KT, GRP], f32, tag=f"ush{b}", name=f"ush{b}")
                   for b in range(2)]
            # Pool-chain scratch for pair 0 (w and clamp intermediates)
            pw = st.tile([TROWS, GRP], f32, tag="pw", name="pw")
            pcl = st.tile([TROWS, GRP], f32, tag="pcl", name="pcl")

            # ---- weights: fp32 staging -> f32r ------------------------
            lt1h = wts.tile([CON1, KH], f32, tag="lt1h", name="lt1h")
            lt1t = wts.tile([CON1, KT], f32, tag="lt1t", name="lt1t")
            lt2h = wts.tile([TROWS, KH], f32, tag="lt2h", name="lt2h")
            lt2t = wts.tile([TROWS, KT], f32, tag="lt2t", name="lt2t")
            w1h = wts.tile([CON1, KH], f32r, tag="w1h", name="w1h")
            w1t = wts.tile([CON1, KT], f32r, tag="w1t", name="w1t")
            w2h = wts.tile([TROWS, KH], f32r, tag="w2h", name="w2h")
            w2t = wts.tile([TROWS, KT], f32r, tag="w2t", name="w2t")

            nc.sync.dma_start(lt1h[:], d_w1h[:])
            nc.sync.dma_start(lt1t[:], d_w1t[:])
            nc.sync.dma_start(lt2h[:], d_w2h[:])
            nc.sync.dma_start(lt2t[:], d_w2t[:])
            nc.scalar.copy(w1h[:], lt1h[:])
            nc.scalar.copy(w1t[:], lt1t[:])
            nc.scalar.copy(w2h[:], lt2h[:])
            nc.scalar.copy(w2t[:], lt2t[:])

            # ---- init: staging holds zeros + Y; engine copies round to
            # f32r in-lane (engine partition offsets must be 32-aligned).
            with tc.tile_pool(name="init", bufs=1) as ip:
                ystage = ip.tile([CON1, NCOLS], f32, tag="ystage",
                                 name="ystage")
                nc.gpsimd.memset(ystage[0:KH, :], 0.0)
                nc.sync.dma_start(ystage[KH:CON1, :], ycols[:, :])
                nc.vector.tensor_copy(xH[0][:, :], ystage[:, :])
                nc.scalar.copy(xH[1][:, :], ystage[:, :])
                nc.gpsimd.tensor_copy(xT[0][0:TROWS, :],
                                      ystage[0:TROWS, 0:TCOLS])
                nc.gpsimd.tensor_copy(tg4[0:TROWS, :],
                                      ystage[0:TROWS, 0:GRP])

            def mm(ps, lhsT, rhs, start, stop):
                nc.tensor.matmul(ps, lhsT, rhs, start=start, stop=stop)

            with tc.tile_pool(name="wp", bufs=2) as wp:
              for it in range(MAX_ITER):
                m_prev = ms[it - 1] if it > 0 else 0.0
                s0 = float(1.0 + m_prev)
                s1 = float(-m_prev)
                xcH, xnH = xH[it % 2], xH[(it + 1) % 2]
                xcT, xnT = xT[it % 2], xT[(it + 1) % 2]
                upc, upp = upT[it % 2], upT[(it + 1) % 2]
                last = it == MAX_ITER - 1

                # stream this iteration's scaled dual-x weight set
                ws = wp.tile([CON1, W4C], f32, tag="ws", name="ws")
                wf = wp.tile([CON1, W4C], f32r, tag="wf", name="wf")
                nc.sync.dma_start(ws[:], d_w4[:, it * W4C:(it + 1) * W4C])
                nc.gpsimd.tensor_copy(wf[:], ws[:])

                pending = None
                for g in range(NGRP - 1):
                    gs = slice(g * GRP, (g + 1) * GRP)
                    par = g % 2
                    pairno = g // 2
                    fb = pairno * GRP            # folded tail col base
                    ts = slice(fb, fb + GRP)
                    rT = slice(0, KT) if par == 0 else slice(FOLD, TROWS)

                    wh = psH.tile([KH, GRP], mybir.dt.float32, tag="wh",
                                  name="wh")
                    wt = psT.tile([KT, GRP], mybir.dt.float32, tag="wt",
                                  name="wt")

                    for b in range(GRP // BLK):
                        bs = slice(g * GRP + b * BLK, g * GRP + (b + 1) * BLK)
                        ps = slice(b * BLK, (b + 1) * BLK)
                        fs = slice(fb + b * BLK, fb + (b + 1) * BLK)
                        mm(wh[:, ps], w1h[:], xcH[:, bs], True, False)
                        mm(wh[:, ps], w2h[rT, :], xcT[rT, fs], False, True)
                        mm(wt[:, ps], w1t[:], xcH[:, bs], True, False)
                        mm(wt[:, ps], w2t[rT, :], xcT[rT, fs], False, True)

                    # emit the previous pair's fused op only now, so it
                    # never heads the in-order DVE queue while its fold
                    # copies are still in flight
                    if pending is not None:
                        pending()
                        pending = None

                    # ---- head pack (PSUM in0 + ACT-scaled history) ------
                    if it == 0:
                        nc.vector._custom_dve(op_first, out=xnH[0:KH, gs],
                                              in0=wh[:], s0=s0,
                                              imm2=float(lam))
                    else:
                        nc.vector._custom_dve(op_mom, out=xnH[0:KH, gs],
                                              in0=wh[:],
                                              in1=uoH[(it + 1) % 2][:, gs],
                                              s0=s0, imm2=float(lam))
                    if last:
                        nc.sync.dma_start(out[0:KH, gs],
                                          xnH[0:KH, gs].bitcast(f32))
                    else:
                        nc.scalar.mul(uoH[it % 2][:, gs], wh[:],
                                      float(-ms[it]))

                    # fold raw tail u into the pair tile: even groups land
                    # at base 0 directly; odd groups go via SBUF staging +
                    # partition-shifting DMA
                    if par == 0:
                        nc.scalar.copy(upc[0:KT, ts], wt[:])
                    else:
                        nc.scalar.copy(ush[pairno][:], wt[:])
                        nc.sync.dma_start(upc[FOLD:TROWS, ts],
                                          ush[pairno][:])

                        def pair_fused(ts=ts, g=g, gs=gs, pairno=pairno):
                            if it == 0:
                                nc.vector._custom_dve(
                                    op_first, out=xnT[0:TROWS, ts],
                                    in0=upc[0:TROWS, ts], s0=s0,
                                    imm2=float(lam))
                            else:
                                nc.vector._custom_dve(
                                    op_mom2, out=xnT[0:TROWS, ts],
                                    in0=upc[0:TROWS, ts],
                                    in1=upp[0:TROWS, ts],
                                    s0=s0, s1=s1, imm2=float(lam))
                            if last:
                                nc.sync.dma_start(
                                    out[KH:K, (g - 1) * GRP:g * GRP],
                                    xnT[0:KT, ts].bitcast(f32))
                                nc.sync.dma_start(
                                    out[KH:K, gs],
                                    xnT[FOLD:TROWS, ts].bitcast(f32))
                        pending = pair_fused

                # ---- group 4: dual-x (w formed fully in PSUM; no history
                # passes).  Contract over [x_i-head+Y | dual tail | x_{i-1}
                # head] with per-iteration scaled weights.  tg4 rows 0:43
                # always hold x_i, rows 64:107 hold x_{i-1}: the DVE is
                # lane-locked so it writes the cur slot, and a partition-
                # shifting DMA refreshes the prev slot after k2 reads it.
                g = G4
                gs = slice(g * GRP, (g + 1) * GRP)
                xpH = xH[(it + 1) % 2]       # holds x_{i-1} before DVE write

                wh = psH.tile([KH, GRP], mybir.dt.float32, tag="wh",
                              name="wh")
                wt = psT.tile([KT, GRP], mybir.dt.float32, tag="wt",
                              name="wt")
                for b in range(GRP // BLK):
                    bs = slice(g * GRP + b * BLK, g * GRP + (b + 1) * BLK)
                    ps = slice(b * BLK, (b + 1) * BLK)
                    ls = slice(b * BLK, (b + 1) * BLK)
                    mm(wh[:, ps], wf[:, 0:KH], xcH[:, bs], True, False)
                    mm(wh[:, ps], wf[0:TROWS, K:K + KH], tg4[:, ls],
                       False, False)
                    mm(wh[:, ps], wf[0:KH, 2 * K:2 * K + KH], xpH[0:KH, bs],
                       False, True)
                    mm(wt[:, ps], wf[:, KH:K], xcH[:, bs], True, False)
                    mm(wt[:, ps], wf[0:TROWS, K + KH:2 * K], tg4[:, ls],
                       False, False)
                    mm(wt[:, ps], wf[0:KH, 2 * K + KH:3 * K], xpH[0:KH, bs],
                       False, True)

                nc.sync.dma_start(tg4[FOLD:TROWS, :], tg4[0:KT, :])
                if pending is not None:
                    pending()
                    pending = None
                nc.vector._custom_dve(op_first, out=xnH[0:KH, gs],
                                      in0=wh[:], s0=1.0, imm2=float(lam))
                nc.vector._custom_dve(op_first, out=tg4[0:KT, :],
                                      in0=wt[:], s0=1.0, imm2=float(lam))
                if last:
                    nc.sync.dma_start(out[0:KH, gs],
                                      xnH[0:KH, gs].bitcast(f32))
                    nc.sync.dma_start(out[KH:K, gs],
                                      tg4[0:KT, :].bitcast(f32))
    nc.finalize()
    return nc


def _get_program(lam, ms):
    key = (round(lam, 12), tuple(round(m, 9) for m in ms))
    if _cache.get("key") != key:
        _cache["consts_meta"] = (lam, ms)
        _cache["nc"] = _build_program()
        _cache["key"] = key
    return _cache["nc"]


# ------------------------------------------------------------------- kernel
def kernel(x, Drr, Dtheta):
    from concourse.bass_utils import run_bass_kernel_spmd

    Aaug, lam, ms = _host_constants(Drr, Dtheta)
    nc = _get_program(lam, ms)

    # contraction row order: [x_head(0:118); Y(161:171) | x_tail(118:161)]
    k1 = np.ascontiguousarray(Aaug[np.r_[0:KH, K:K + T]])   # [128, 161]
    k2 = Aaug[KH:K]                                          # [43, 161]
    w1h = np.ascontiguousarray(k1[:, 0:KH])
    w1t = np.ascontiguousarray(k1[:, KH:K])
    w2h = np.zeros((TROWS, KH), np.float32)
    w2t = np.zeros((TROWS, KT), np.float32)
    w2h[0:KT] = k2[:, 0:KH]
    w2h[FOLD:TROWS] = k2[:, 0:KH]
    w2t[0:KT] = k2[:, KH:K]
    w2t[FOLD:TROWS] = k2[:, KH:K]

    # group-4 dual-x weight sets: [k1 | k2 | k3] per iteration, pre-scaled
    # by the momentum coefficients of that iteration
    w4 = np.zeros((CON1, W4C * MAX_ITER), np.float32)
    Ah = Aaug[0:KH]          # x-head rows
    At = Aaug[KH:K]          # x-tail rows
    Ay = Aaug[K:K + T]       # Y rows
    for it in range(MAX_ITER):
        m_prev = ms[it - 1] if it > 0 else 0.0
        a, bb = 1.0 + m_prev, -m_prev
        ce, co = a, bb          # rows 0:43 = x_i, rows 64:107 = x_{i-1}
        o = it * W4C
        w4[0:KH, o:o + K] = a * Ah
        w4[KH:CON1, o:o + K] = Ay
        w4[0:KT, o + K:o + 2 * K] = ce * At
        w4[FOLD:TROWS, o + K:o + 2 * K] = co * At
        w4[0:KH, o + 2 * K:o + 3 * K] = bb * Ah

    xc = np.ascontiguousarray(
        np.transpose(x.astype(np.float32), (1, 0, 2)).reshape(T, B * P))

    in_maps = []
    for c in range(NCORES):
        in_maps.append({
            "ycols": np.ascontiguousarray(xc[:, c * NCOLS:(c + 1) * NCOLS]),
            "w1h": w1h, "w1t": w1t, "w2h": w2h, "w2t": w2t, "w4": w4,
        })

    res = run_bass_kernel_spmd(nc, in_maps, core_ids=list(range(NCORES)))
    _cache["last_res"] = res
    full = np.concatenate([r["out"] for r in res.results], axis=1)  # [K, B*P]
    return np.ascontiguousarray(
        full.reshape(K, B, P).transpose(1, 0, 2)).astype(np.float32)


if __name__ == "__main__":
    x = np.random.randn(B, T, P).astype(np.float32)
    Drr = np.random.rand(N_POLES).astype(np.float32)
    Dtheta = np.random.rand(N_POLES).astype(np.float32)
    o = kernel(x, Drr, Dtheta)
    print(o.shape, o.dtype)


# revision 31
# speedup vs baseline: 1.4606x; 1.4606x over previous
"""FISTA sparse-coding encoder kernel for Trainium2 (8 NeuronCores).

Problem: x [2,10,20480] f32, Drr/Dtheta [40] f32.
  D = normalized dictionary [10, 161]
  A = I - D^T D / L,  DtY = D^T Y / L,  lam = gamma / L
  40 FISTA iterations: xn = softshrink(A @ y + DtY); y = xn + m (xn - x_old)
  output sparsecode [2, 161, 20480].

Design ("fold-8"):
  - Data-parallel over columns: Y reshaped to [10, 40960]; 5120 columns/core.
  - u-form + momentum identity: u_i = A x_i + DtY;
    A y_i + DtY = (1+m) u_i - m u_{i-1}.
  - Output-row split [118 | 43]; Y is glued into the HEAD contraction k-tile
    (state rows 118:128), so the head DVE write (rows 0:118) never touches it.
  - Tail folding: group pairs (2p, 2p+1) share one elementwise pack. Matmul
    PSUM stays at partition base 0 (column-tiled PE writes are rejected by
    walrus); the fold happens via DMA: raw tail u is DMA-evacuated into a
    ping-pong SBUF pair tile at partition offsets {0, 64}. One 7-stage DVE
    op (w = C0*u + C1*u_prev, then clamp-shrink) covers both groups.
    => 8 elementwise packs/iter instead of 10, and the pair packs carry
    their own momentum history (no ACT work for them).
  - Heads + last odd group: classic path (PSUM in0 + ACT-evacuated -m*u).
  - Tail state is column-folded [107, 3072]; odd groups' matmul operands
    (weights dup + state) live at partition base 64 (row-tiled PE reads).
  - State zero-initialised => iteration 0 is a regular iteration
    (u_0 = A*0 + DtY) with a Src1-less fused op; no special weights.
  - All matmul operands float32r (full-rate fp32 PE path).
"""

import numpy as np

# ---------------------------------------------------------------- constants
B, T, N_POLES, P = 2, 10, 40, 20480
MAX_ITER = 40
GAMMA = 0.01
K = 4 * N_POLES + 1          # 161
NCORES = 8
NCOLS = B * P // NCORES      # 5120 columns per core
BLK = 512                    # matmul free dim (one PSUM bank)
GRP = 1024                   # elementwise pack width (2 banks)
NGRP = NCOLS // GRP          # 5
KH = 118                     # head output rows
KT = K - KH                  # 43 tail output rows
CON1 = KH + T                # 128 = head k-tile: x_head(118) + Y(10)
FOLD = 64                    # partition offset of odd-group tails
TROWS = FOLD + KT            # 107 rows in folded tail tiles
NPAIR = NGRP // 2            # 2 full pairs; group 4 is dual-x
TCOLS = NPAIR * GRP          # 2048 folded tail columns (pairs only)
G4 = NGRP - 1                # the dual-x group
W4C = 3 * K                  # 483 = per-iteration dual-x weight columns

_cache = {}


# ------------------------------------------------------------ custom DVE ops
def _register_shrink_ops():
    """shrink(w) = w - clamp(w, -C2, C2).

    op_mom   : w = in0*s0 + in1          (in1 pre-scaled by ACT: -m*u_prev)
    op_mom2  : w = in0*s0 + in1*s1       (both raw u copies in SBUF)
    op_first : w = in0*s0                (iteration 0)
    """
    import concourse.dve_ops as dve_ops
    from concourse.dve_spec import Spec, Src0, Src1, C0, C1, C2, Zero, minn, maxx, lower
    from concourse.dve_spec import _has_src1 as has_src1
    from concourse.dve_uop import DveOpSpec

    def reg(name, body, ref):
        if any(op.name == name for op in dve_ops.OPS):
            return next(op for op in dve_ops.OPS if op.name == name)
        spec = Spec(body=body, reference=ref)
        op = dve_ops.DveOp(name, spec, subdim=False, uops_sha={})
        dve_ops.OPS.append(op)
        dve_ops.CUSTOM_DVE_SPECS[name] = spec
        dve_ops._SUB_OPCODE_FOR_NAME[name] = (
            dve_ops._CUSTOM_DVE_ROW_BASE + len(dve_ops.OPS) - 1
        )
        for ver in ("v3", "v4"):
            compiled = DveOpSpec(
                name=name,
                opcode=dve_ops.get_dve_sub_opcode(name),
                uops=lower(spec, ver=ver),
                rd1_en=has_src1(spec),
            )
            op.uops_sha[ver] = compiled.sha(ver)
        return op

    def shrink_ref(ww, imm2):
        return (ww - np.maximum(np.minimum(ww, imm2), -imm2)).astype(np.float32)

    w1 = Src0 * C0 + Src1
    op_mom = reg(
        "ANT_SHRINKCLAMP_MOM",
        w1 - maxx(minn(w1, C2), Zero - C2),
        lambda in0, in1, s0=1.0, s1=0.0, imm2=0.0: shrink_ref(in0 * s0 + in1, imm2),
    )
    w2 = Src0 * C0 + Src1 * C1
    op_mom2 = reg(
        "ANT_SHRINKCLAMP_MOM2",
        w2 - maxx(minn(w2, C2), Zero - C2),
        lambda in0, in1, s0=1.0, s1=0.0, imm2=0.0: shrink_ref(
            in0 * s0 + in1 * s1, imm2),
    )
    w0 = Src0 * C0
    op_first = reg(
        "ANT_SHRINKCLAMP_FIRST",
        w0 - maxx(minn(w0, C2), Zero - C2),
        lambda in0, s0=1.0, s1=0.0, imm2=0.0: shrink_ref(in0 * s0, imm2),
    )
    return op_mom, op_mom2, op_first


# ------------------------------------------------------------ host constants
def _host_constants(Drr, Dtheta):
    r = Drr.astype(np.float64)
    th = Dtheta.astype(np.float64)
    i = np.arange(T, dtype=np.float64)[:, None]
    pr = r[None, :] ** i
    sgn = np.where(np.arange(T)[:, None] % 2 == 0, 1.0, -1.0)
    c = np.cos(i * th[None, :])
    s = np.sin(i * th[None, :])
    ones = np.ones((T, 1))
    dic = np.concatenate([ones, pr * c, sgn * pr * c, pr * s, sgn * pr * s], axis=1)
    G = np.linalg.norm(dic, axis=0)
    G = np.where(G == 0, np.sqrt(float(T)), G)
    D = dic / G                                 # [T, K] float64

    DtD = D.T @ D
    L = float(np.linalg.norm(DtD))              # Frobenius
    A = np.eye(K) - DtD / L                     # [K, K]
    lam = float(GAMMA / L)

    Aaug = np.concatenate([A.T, D / L], axis=0).astype(np.float32)  # [171, K]

    ms = []
    t = 1.0
    for _ in range(MAX_ITER):
        t_new = (1.0 + np.sqrt(1.0 + 4.0 * t * t)) / 2.0
        ms.append((t - 1.0) / t_new)
        t = t_new
    return Aaug, lam, ms


# ------------------------------------------------------------- bass program
def _build_program():
    import concourse.mybir as mybir
    import concourse.tile as tile
    from concourse import bacc

    op_mom, op_mom2, op_first = _register_shrink_ops()

    f32 = mybir.dt.float32
    f32r = mybir.dt.float32r

    nc = bacc.Bacc("TRN2", target_bir_lowering=False, debug=False,
                   num_devices=NCORES)

    ycols = nc.dram_tensor("ycols", [T, NCOLS], f32, kind="ExternalInput")
    d_w1h = nc.dram_tensor("w1h", [CON1, KH], f32, kind="ExternalInput")
    d_w1t = nc.dram_tensor("w1t", [CON1, KT], f32, kind="ExternalInput")
    d_w2h = nc.dram_tensor("w2h", [TROWS, KH], f32, kind="ExternalInput")
    d_w2t = nc.dram_tensor("w2t", [TROWS, KT], f32, kind="ExternalInput")
    d_w4 = nc.dram_tensor("w4", [CON1, W4C * MAX_ITER], f32,
                          kind="ExternalInput")
    out = nc.dram_tensor("out", [K, NCOLS], f32, kind="ExternalOutput")

    lam, ms = _cache["consts_meta"]

    with tile.TileContext(nc) as tc:
        with (
            tc.tile_pool(name="state", bufs=1) as st,
            tc.tile_pool(name="wts", bufs=1) as wts,
            tc.tile_pool(name="psH", bufs=2, space="PSUM") as psH,
            tc.tile_pool(name="psT", bufs=2, space="PSUM") as psT,
        ):
            # ---- persistent state -------------------------------------
            # xH rows 0:118 = head state, rows 118:128 = Y glue (static)
            xH = [st.tile([CON1, NCOLS], f32r, tag=f"xH{b}", name=f"xH{b}")
                  for b in range(2)]
            # folded tail state: rows 0:43 even groups, 64:107 odd groups
            xT = [st.tile([TROWS, TCOLS], f32r, tag=f"xT{b}", name=f"xT{b}")
                  for b in range(2)]
            # double-buffered so the evacuation never waits on the fused op
            uoH = [st.tile([KH, NCOLS], f32, tag=f"uoH{b}", name=f"uoH{b}")
                   for b in range(2)]
            # group-4 dual-x tail state: x_i at rows 0:43 (even i) or
            # 64:107 (odd i); the other slot holds x_{i-1}
            tg4 = st.tile([TROWS, GRP], f32r, tag="tg4", name="tg4")
            # raw tail-u pair copies (ping-pong momentum history)
            upT = [st.tile([TROWS, NPAIR * GRP], f32, tag=f"upT{b}",
                           name=f"upT{b}") for b in range(2)]
            # staging for the odd-group partition shift (engine writes are
            # lane-locked; only DMA can move partitions 0:43 -> 64:107)
            ush = [st.tile([KT, GRP], f32, tag=f"ush{b}", name=f"ush{b}")
                   for b in range(2)]
            # Pool-chain scratch for pair 0 (w and clamp intermediates)
            pw = st.tile([TROWS, GRP], f32, tag="pw", name="pw")
            pcl = st.tile([TROWS, GRP], f32, tag="pcl", name="pcl")

            # ---- weights: fp32 staging -> f32r ------------------------
            lt1h = wts.tile([CON1, KH], f32, tag="lt1h", name="lt1h")
            lt1t = wts.tile([CON1, KT], f32, tag="lt1t", name="lt1t")
            lt2h = wts.tile([TROWS, KH], f32, tag="lt2h", name="lt2h")
            lt2t = wts.tile([TROWS, KT], f32, tag="lt2t", name="lt2t")
            w1h = wts.tile([CON1, KH], f32r, tag="w1h", name="w1h")
            w1t = wts.tile([CON1, KT], f32r, tag="w1t", name="w1t")
            w2h = wts.tile([TROWS, KH], f32r, tag="w2h", name="w2h")
            w2t = wts.tile([TROWS, KT], f32r, tag="w2t", name="w2t")

            nc.sync.dma_start(lt1h[:], d_w1h[:])
            nc.sync.dma_start(lt1t[:], d_w1t[:])
            nc.sync.dma_start(lt2h[:], d_w2h[:])
            nc.sync.dma_start(lt2t[:], d_w2t[:])
            nc.scalar.copy(w1h[:], lt1h[:])
            nc.scalar.copy(w1t[:], lt1t[:])
            nc.scalar.copy(w2h[:], lt2h[:])
            nc.scalar.copy(w2t[:], lt2t[:])

            # ---- init: staging holds zeros + Y; engine copies round to
            # f32r in-lane (engine partition offsets must be 32-aligned).
            with tc.tile_pool(name="init", bufs=1) as ip:
                ystage = ip.tile([CON1, NCOLS], f32, tag="ystage",
                                 name="ystage")
                nc.gpsimd.memset(ystage[0:KH, :], 0.0)
                nc.sync.dma_start(ystage[KH:CON1, :], ycols[:, :])
                nc.vector.tensor_copy(xH[0][:, :], ystage[:, :])
                nc.scalar.copy(xH[1][:, :], ystage[:, :])
                nc.gpsimd.tensor_copy(xT[0][0:TROWS, :],
                                      ystage[0:TROWS, 0:TCOLS])
                nc.gpsimd.tensor_copy(tg4[0:TROWS, :],
                                      ystage[0:TROWS, 0:GRP])

            def mm(ps, lhsT, rhs, start, stop):
                nc.tensor.matmul(ps, lhsT, rhs, start=start, stop=stop)

            with tc.tile_pool(name="wp", bufs=2) as wp:
              for it in range(MAX_ITER):
                m_prev = ms[it - 1] if it > 0 else 0.0
                s0 = float(1.0 + m_prev)
                s1 = float(-m_prev)
                xcH, xnH = xH[it % 2], xH[(it + 1) % 2]
                xcT, xnT = xT[it % 2], xT[(it + 1) % 2]
                upc, upp = upT[it % 2], upT[(it + 1) % 2]
                last = it == MAX_ITER - 1

                # stream this iteration's scaled dual-x weight set
                ws = wp.tile([CON1, W4C], f32, tag="ws", name="ws")
                wf = wp.tile([CON1, W4C], f32r, tag="wf", name="wf")
                nc.sync.dma_start(ws[:], d_w4[:, it * W4C:(it + 1) * W4C])
                nc.gpsimd.tensor_copy(wf[:], ws[:])

                pending = None
                for g in range(NGRP - 1):
                    gs = slice(g * GRP, (g + 1) * GRP)
                    par = g % 2
                    pairno = g // 2
                    fb = pairno * GRP            # folded tail col base
                    ts = slice(fb, fb + GRP)
                    rT = slice(0, KT) if par == 0 else slice(FOLD, TROWS)

                    wh = psH.tile([KH, GRP], mybir.dt.float32, tag="wh",
                                  name="wh")
                    wt = psT.tile([KT, GRP], mybir.dt.float32, tag="wt",
                                  name="wt")

                    for b in range(GRP // BLK):
                        bs = slice(g * GRP + b * BLK, g * GRP + (b + 1) * BLK)
                        ps = slice(b * BLK, (b + 1) * BLK)
                        fs = slice(fb + b * BLK, fb + (b + 1) * BLK)
                        mm(wh[:, ps], w1h[:], xcH[:, bs], True, False)
                        mm(wh[:, ps], w2h[rT, :], xcT[rT, fs], False, True)
                        mm(wt[:, ps], w1t[:], xcH[:, bs], True, False)
                        mm(wt[:, ps], w2t[rT, :], xcT[rT, fs], False, True)

                    # emit the previous pair's fused op only now, so it
                    # never heads the in-order DVE queue while its fold
                    # copies are still in flight
                    if pending is not None:
                        pending()
                        pending = None

                    # ---- head pack (PSUM in0 + ACT-scaled history) ------
                    if it == 0:
                        nc.vector._custom_dve(op_first, out=xnH[0:KH, gs],
                                              in0=wh[:], s0=s0,
                                              imm2=float(lam))
                    else:
                        nc.vector._custom_dve(op_mom, out=xnH[0:KH, gs],
                                              in0=wh[:],
                                              in1=uoH[(it + 1) % 2][:, gs],
                                              s0=s0, imm2=float(lam))
                    if last:
                        nc.sync.dma_start(out[0:KH, gs],
                                          xnH[0:KH, gs].bitcast(f32))
                    else:
                        nc.scalar.mul(uoH[it % 2][:, gs], wh[:],
                                      float(-ms[it]))

                    # fold raw tail u into the pair tile: even groups land
                    # at base 0 directly; odd groups go via SBUF staging +
                    # partition-shifting DMA
                    if par == 0:
                        nc.scalar.copy(upc[0:KT, ts], wt[:])
                    else:
                        nc.scalar.copy(ush[pairno][:], wt[:])
                        nc.sync.dma_start(upc[FOLD:TROWS, ts],
                                          ush[pairno][:])

                        def pair_fused(ts=ts, g=g, gs=gs, pairno=pairno):
                            if it == 0:
                                nc.vector._custom_dve(
                                    op_first, out=xnT[0:TROWS, ts],
                                    in0=upc[0:TROWS, ts], s0=s0,
                                    imm2=float(lam))
                            else:
                                nc.vector._custom_dve(
                                    op_mom2, out=xnT[0:TROWS, ts],
                                    in0=upc[0:TROWS, ts],
                                    in1=upp[0:TROWS, ts],
                                    s0=s0, s1=s1, imm2=float(lam))
                            if last:
                                nc.sync.dma_start(
                                    out[KH:K, (g - 1) * GRP:g * GRP],
                                    xnT[0:KT, ts].bitcast(f32))
                                nc.sync.dma_start(
                                    out[KH:K, gs],
                                    xnT[FOLD:TROWS, ts].bitcast(f32))
                        pending = pair_fused

                # ---- group 4: dual-x (w formed fully in PSUM; no history
                # passes).  Contract over [x_i-head+Y | dual tail | x_{i-1}
                # head] with per-iteration scaled weights.  tg4 rows 0:43
                # always hold x_i, rows 64:107 hold x_{i-1}: the DVE is
                # lane-locked so it writes the cur slot, and a partition-
                # shifting DMA refreshes the prev slot after k2 reads it.
                g = G4
                gs = slice(g * GRP, (g + 1) * GRP)
                xpH = xH[(it + 1) % 2]       # holds x_{i-1} before DVE write

                wh = psH.tile([KH, GRP], mybir.dt.float32, tag="wh",
                              name="wh")
                wt = psT.tile([KT, GRP], mybir.dt.float32, tag="wt",
                              name="wt")
                for b in range(GRP // BLK):
                    bs = slice(g * GRP + b * BLK, g * GRP + (b + 1) * BLK)
                    ps = slice(b * BLK, (b + 1) * BLK)
                    ls = slice(b * BLK, (b + 1) * BLK)
                    mm(wh[:, ps], wf[:, 0:KH], xcH[:, bs], True, False)
                    mm(wh[:, ps], wf[0:TROWS, K:K + KH], tg4[:, ls],
                       False, False)
                    mm(wh[:, ps], wf[0:KH, 2 * K:2 * K + KH], xpH[0:KH, bs],
                       False, True)
                    mm(wt[:, ps], wf[:, KH:K], xcH[:, bs], True, False)
                    mm(wt[:, ps], wf[0:TROWS, K + KH:2 * K], tg4[:, ls],
                       False, False)
                    mm(wt[:, ps], wf[0:KH, 2 * K + KH:3 * K], xpH[0:KH, bs],
                       False, True)

                nc.sync.dma_start(tg4[FOLD:TROWS, :], tg4[0:KT, :])
                if pending is not None:
                    pending()
                    pending = None
                nc.vector._custom_dve(op_first, out=xnH[0:KH, gs],
                                      in0=wh[:], s0=1.0, imm2=float(lam))
                nc.vector._custom_dve(op_first, out=tg4[0:KT, :],
                                      in0=wt[:], s0=1.0, imm2=float(lam))
                if last:
                    nc.sync.dma_start(out[0:KH, gs],
                                      xnH[0:KH, gs].bitcast(f32))
                    nc.sync.dma_start(out[KH:K, gs],
                                      tg4[0:KT, :].bitcast(f32))
    nc.finalize()
    return nc


def _get_program(lam, ms):
    key = (round(lam, 12), tuple(round(m, 9) for m in ms))
    if _cache.get("key") != key:
        _cache["consts_meta"] = (lam, ms)
        _cache["nc"] = _build_program()
        _cache["key"] = key
    return _cache["nc"]


# ------------------------------------------------------------------- kernel
def kernel(x, Drr, Dtheta):
    from concourse.bass_utils import run_bass_kernel_spmd

    Aaug, lam, ms = _host_constants(Drr, Dtheta)
    nc = _get_program(lam, ms)

    # contraction row order: [x_head(0:118); Y(161:171) | x_tail(118:161)]
    k1 = np.ascontiguousarray(Aaug[np.r_[0:KH, K:K + T]])   # [128, 161]
    k2 = Aaug[KH:K]                                          # [43, 161]
    w1h = np.ascontiguousarray(k1[:, 0:KH])
    w1t = np.ascontiguousarray(k1[:, KH:K])
    w2h = np.zeros((TROWS, KH), np.float32)
    w2t = np.zeros((TROWS, KT), np.float32)
    w2h[0:KT] = k2[:, 0:KH]
    w2h[FOLD:TROWS] = k2[:, 0:KH]
    w2t[0:KT] = k2[:, KH:K]
    w2t[FOLD:TROWS] = k2[:, KH:K]

    # group-4 dual-x weight sets: [k1 | k2 | k3] per iteration, pre-scaled
    # by the momentum coefficients of that iteration
    w4 = np.zeros((CON1, W4C * MAX_ITER), np.float32)
    Ah = Aaug[0:KH]          # x-head rows
    At = Aaug[KH:K]          # x-tail rows
    Ay = Aaug[K:K + T]       # Y rows
    for it in range(MAX_ITER):
        m_prev = ms[it - 1] if it > 0 else 0.0
        a, bb = 1.0 + m_prev, -m_prev
        ce, co = a, bb          # rows 0:43 = x_i, rows 64:107 = x_{i-1}
        o = it * W4C
        w4[0:KH, o:o + K] = a * Ah
        w4[KH:CON1, o:o + K] = Ay
        w4[0:KT, o + K:o + 2 * K] = ce * At
        w4[FOLD:TROWS, o + K:o + 2 * K] = co * At
        w4[0:KH, o + 2 * K:o + 3 * K] = bb * Ah

    xc = np.ascontiguousarray(
        np.transpose(x.astype(np.float32), (1, 0, 2)).reshape(T, B * P))

    in_maps = []
    for c in range(NCORES):
        in_maps.append({
            "ycols": np.ascontiguousarray(xc[:, c * NCOLS:(c + 1) * NCOLS]),
            "w1h": w1h, "w1t": w1t, "w2h": w2h, "w2t": w2t, "w4": w4,
        })

    res = run_bass_kernel_spmd(nc, in_maps, core_ids=list(range(NCORES)))
    _cache["last_res"] = res
    full = np.concatenate([r["out"] for r in res.results], axis=1)  # [K, B*P]
    return np.ascontiguousarray(
        full.reshape(K, B, P).transpose(1, 0, 2)).astype(np.float32)


if __name__ == "__main__":
    x = np.random.randn(B, T, P).astype(np.float32)
    Drr = np.random.rand(N_POLES).astype(np.float32)
    Dtheta = np.random.rand(N_POLES).astype(np.float32)
    o = kernel(x, Drr, Dtheta)
    print(o.shape, o.dtype)
